# revision 3
# baseline (speedup 1.0000x reference)
"""Trainium2 kernel for nn_Linter_89000312307760 (segment_reduce).

Pipeline (no host-side sort / layout shuffle — the 128 MB feature tensor is
touched exactly once on host, for the fp16 cast):
  host:   key = m*label + index per pixel (int32 -> fp16, exact for <2048);
          counts via bincount; feat cast f32->fp16 with torch (fast).
  device: (8 cores, data-parallel: core = image*4 + quarter)
          - XBAR DMA-transpose streams feat [64, 65536] fp16 into pixel-major
            tiles [128, 64] (free: done by the DMA engines, no compute cost)
          - one-hot per pixel-tile via tensor_scalar(iota == key) on POOL/DVE
          - segment sums via PSUM-accumulated matmuls: psum[64, 641] +=
            featT[128, 64]^T x onehot[128, 641] over all 512 tiles
          - one [64, 641] f32 output per core.
  host:   sum the 4 quarter outputs per image; per-class pairwise mean-|.|
          loss via an O(D n log n) sorted-prefix-sum identity (not the O(n^2 D)
          pairwise matrix); final -log scalar.

The Bass module is input-independent: it is built once at import and reused
across calls; the first dispatch is warmed at import as well.
"""
import os
import sys
import time

import numpy as np

if "/opt/trn_rl_repo" not in sys.path:
    sys.path.insert(0, "/opt/trn_rl_repo")

import bass_rust
import concourse.bass as bass
import concourse.tile as tile
from concourse import mybir
from concourse.bass_utils import run_bass_kernel_spmd
from concourse.vector_clock import ScopedClock

# ---- problem constants (hardcoded per spec) ----
B, D, H, W = 2, 64, 512, 512
P = H * W                    # pixels per image
N_CLASSES = 5
IGNORE_LB = 255
S = N_CLASSES * 128 + 1      # 641 static segment capacity
N_CORES = 8
QUARTER = P // 4             # pixels per core chunk
N_TILES = QUARTER // 128     # 512 pixel-tiles per core
CHUNK_TILES = 32             # tiles per transpose-DMA chunk

LAST_RUN_WALL_S = None       # wall-clock of the device execute (set per call)


# ---------------------------------------------------------------- drain patch
def _patched_drain_and_barrier(self, tick_clock, wait_clock):
    # walrus CTRL ops encode only one sync wait; the stock kernel-tail drain
    # carries one wait per logical processor. Spread them over SP nops.
    nc = self.nc
    probe = nc.sync.nop(nofuse=True, hint="drain_wait_probe")
    wait_clock.add_sem_waits(probe.ins, ScopedClock({None: tick_clock.global_clock}))
    waits = list(probe.ins.sync_info.on_wait) if probe.ins.sync_info else []
    if len(waits) > 1:
        probe.ins.sync_info = bass_rust.SyncInfo(on_wait=waits[:1], on_update=[])
        for i, w in enumerate(waits[1:]):
            n = nc.sync.nop(nofuse=True, hint=f"drain_wait_{i}")
            n.ins.sync_info = bass_rust.SyncInfo(on_wait=[w], on_update=[])
    nc.sync.drain()
    nc.all_engine_barrier()
    assert self.sems is not None
    popped = nc._tile_sem_poison_stack.pop()
    assert popped is self._sem_poison
    nc.clear_and_free_semaphores(list(self.sems.allocated().values()))
    nc.all_engine_barrier()


tile.TileContext._drain_and_barrier = _patched_drain_and_barrier

_WSPLIT_N = 0


def _split_sync_waits(nc: bass.Bass):
    """walrus encodes at most one sync wait per instruction on this target;
    move extra waits onto same-engine nops inserted immediately before."""
    global _WSPLIT_N
    for f in nc.m.functions:
        for bb in f.blocks:
            out = []
            changed = False
            for ins in bb.instructions:
                si = ins.sync_info
                if si is not None and si.on_wait and len(si.on_wait) > 1:
                    changed = True
                    waits = list(si.on_wait)
                    for w in waits[:-1]:
                        _WSPLIT_N += 1
                        out.append(
                            mybir.InstNoOp(
                                name=f"WSPLIT-{_WSPLIT_N}",
                                engine=ins.engine,
                                bass_nofuse=True,
                                sync_info=mybir.SyncInfo(on_wait=[w], on_update=[]),
                            )
                        )
                    ins.sync_info = mybir.SyncInfo(
                        on_wait=[waits[-1]], on_update=list(si.on_update)
                    )
                out.append(ins)
            if changed:
                bb.instructions = out


# ---------------------------------------------------------------- device part
def build_device_kernel(
    n_tiles: int = N_TILES,
    chunk_tiles: int = CHUNK_TILES,
    ft_bufs: int = 3,
    oh_bufs: int = 8,
) -> bass.Bass:
    nc = bass.Bass("TRN2")
    f16 = mybir.dt.float16
    f32 = mybir.dt.float32

    feat_d = nc.declare_dram_parameter("feat", [64, n_tiles * 128], f16, isOutput=False)
    keys_d = nc.declare_dram_parameter("keys", [128, n_tiles], f32, isOutput=False)
    iota_d = nc.declare_dram_parameter("iota", [128, S], f16, isOutput=False)
    out_d = nc.declare_dram_parameter("out", [64, S], f32, isOutput=True)

    n_chunks = (n_tiles + chunk_tiles - 1) // chunk_tiles

    with tile.TileContext(nc) as tc:
        with (
            tc.tile_pool(name="const", bufs=1) as const_tp,
            tc.tile_pool(name="ft", bufs=ft_bufs) as ft_tp,
            tc.tile_pool(name="oh", bufs=oh_bufs) as oh_tp,
            tc.tile_pool(name="o", bufs=1) as out_tp,
            tc.tile_pool(name="ps", bufs=1, space="PSUM") as ps_tp,
        ):
            iota_sb = const_tp.tile([128, S], f16)
            nc.sync.dma_start(out=iota_sb[:], in_=iota_d[:])
            keys_sb = const_tp.tile([128, n_tiles], f32)
            nc.sync.dma_start(out=keys_sb[:], in_=keys_d[:])

            psA = ps_tp.tile([64, 512], f32, space="PSUM")
            psB = ps_tp.tile([64, S - 512], f32, space="PSUM")

            for c in range(n_chunks):
                t0 = c * chunk_tiles
                t1 = min(t0 + chunk_tiles, n_tiles)
                ft = ft_tp.tile([128, t1 - t0, 64], f16, tag="ft")
                nc.sync.dma_start_transpose(
                    out=ft[:], in_=feat_d[:, t0 * 128 : t1 * 128]
                )
                for t in range(t0, t1):
                    lt = t - t0
                    oh = oh_tp.tile([128, S], f16, tag="oh")
                    eng = nc.gpsimd if (t % 2 == 0) else nc.vector
                    eng.tensor_scalar(
                        out=oh[:],
                        in0=iota_sb[:],
                        scalar1=keys_sb[:, t : t + 1],
                        scalar2=None,
                        op0=mybir.AluOpType.is_equal,
                    )
                    nc.tensor.matmul(
                        out=psA[:],
                        lhsT=ft[:, lt, :],
                        rhs=oh[:, 0:512],
                        start=(t == 0),
                        stop=(t == n_tiles - 1),
                    )
                    nc.tensor.matmul(
                        out=psB[:],
                        lhsT=ft[:, lt, :],
                        rhs=oh[:, 512:S],
                        start=(t == 0),
                        stop=(t == n_tiles - 1),
                    )

            out_sb = out_tp.tile([64, S], f32)
            nc.scalar.activation(
                out=out_sb[:, 0:512], in_=psA[:],
                func=mybir.ActivationFunctionType.Copy,
            )
            nc.scalar.activation(
                out=out_sb[:, 512:S], in_=psB[:],
                func=mybir.ActivationFunctionType.Copy,
            )
            nc.sync.dma_start(out=out_d[:], in_=out_sb[:])

    _split_sync_waits(nc)
    return nc


_NC = None


def _get_nc() -> bass.Bass:
    global _NC
    if _NC is None:
        _NC = build_device_kernel()
    return _NC


# ------------------------------------------------------------------ host part
_IOTA = None


def _get_iota() -> np.ndarray:
    global _IOTA
    if _IOTA is None:
        _IOTA = np.ascontiguousarray(
            np.broadcast_to(np.arange(S, dtype=np.float16), (128, S))
        )
    return _IOTA


def _host_prep(feature_out, labels, indexes):
    """Keys + counts + fp16 feature cast; builds per-core in_maps (views)."""
    lab = np.asarray(labels).reshape(B, P)
    idx = np.asarray(indexes).reshape(B, P)

    m = idx.max(axis=1)                                   # per-image max index
    ig = lab == IGNORE_LB
    keys = np.where(ig, 0, m[:, None] * np.where(ig, 0, lab) + np.where(ig, 0, idx))
    keys = keys.astype(np.int32)                          # [B, P] in [0, S)
    counts = np.stack([np.bincount(keys[b], minlength=S) for b in range(B)])
    keys32 = keys.astype(np.float32)

    import torch

    feat = np.asarray(feature_out, dtype=np.float32)
    feat16 = torch.from_numpy(feat.reshape(B, D, P)).half().numpy()  # [B, D, P]

    iota = _get_iota()
    in_maps = []
    for core in range(N_CORES):
        b, q = divmod(core, 4)
        lo = q * QUARTER
        in_maps.append(
            {
                "feat": feat16[b][:, lo : lo + QUARTER],
                "keys": keys32[b, lo : lo + QUARTER].reshape(N_TILES, 128).T,
                "iota": iota,
            }
        )
    return in_maps, counts, m


def _phase2(sums_b, counts_b, m_b):
    """Per-image pairwise class loss via sorted prefix sums, O(D n log n).

    For one feature dim d with values v sorted ascending and class labels c:
      sum_{j in class b} |v_k - v_j|
        = v_k*(2*cnt_b(<=k) - tot_b) + sum_b(tot) - 2*sum_b(<=k)
    accumulated into P[class(k), b], then summed over d.

    sums_b: [S, D] f32/f64 segment sums; counts_b: [S] int; m_b: max index.
    """
    cnt = counts_b.astype(np.float64)
    valid = cnt >= 2.0
    valid[0] = False
    iv = np.flatnonzero(valid)
    if iv.size == 0:
        return 0.0, 0.0
    mv = sums_b[iv].astype(np.float64) / cnt[iv, None]    # [n, D] means
    cls = (np.ceil(iv.astype(np.float64) / float(m_b)) - 1.0).astype(np.int64)

    order = np.argsort(mv, axis=0)
    w = np.take_along_axis(mv, order, 0)                  # [n, D] col-sorted
    csort = cls[order]                                    # [n, D]
    masks = [(csort == c) for c in range(N_CLASSES)]

    pair = np.zeros((N_CLASSES, N_CLASSES))
    for b in range(N_CLASSES):
        mb = masks[b]
        cntb = np.cumsum(mb, axis=0)
        sumb = np.cumsum(np.where(mb, w, 0.0), axis=0)
        term = w * (2.0 * cntb - cntb[-1][None, :]) + (sumb[-1][None, :] - 2.0 * sumb)
        for a in range(N_CLASSES):
            pair[a, b] = np.sum(term, where=masks[a])
    pair /= float(D)

    n_c = np.bincount(cls, minlength=N_CLASSES).astype(np.float64)
    npair = np.outer(n_c, n_c)
    ret = pair / np.maximum(npair, 1.0)
    h = np.where(ret < 1.0, 0.5 * ret * ret, ret - 0.5)
    tri = np.triu(np.ones((N_CLASSES, N_CLASSES)), k=1)
    pv = tri * (npair > 0.0)
    return float((h * pv).sum()), float(pv.sum())


def kernel(feature_out, labels, indexes):
    global LAST_RUN_WALL_S
    in_maps, counts, m = _host_prep(feature_out, labels, indexes)

    nc = _get_nc()
    t0 = time.monotonic()
    res = run_bass_kernel_spmd(nc, in_maps, core_ids=list(range(N_CORES)))
    LAST_RUN_WALL_S = time.monotonic() - t0

    tot_s = tot_c = 0.0
    for b in range(B):
        sums = (
            res.results[4 * b + 0]["out"].astype(np.float64)
            + res.results[4 * b + 1]["out"]
            + res.results[4 * b + 2]["out"]
            + res.results[4 * b + 3]["out"]
        ).T                                               # [S, D]
        s_img, c_img = _phase2(sums, counts[b], int(m[b]))
        tot_s += s_img
        tot_c += c_img

    mean_h = tot_s / max(tot_c, 1.0)
    mean_h = max(mean_h, 1e-12)
    out = -np.log(mean_h / float(B)) if tot_c > 0 else 0.0
    return np.array([out], dtype=np.float32)


# Build the module at import; warm the dispatch path (jit trace + NEFF cache +
# program load) so the first timed kernel() call pays only transfer + exec.
if os.environ.get("KERNEL_NO_WARM", "") != "1":
    try:
        _get_nc()
        _zero_maps = [
            {
                "feat": np.zeros((64, QUARTER), np.float16),
                "keys": np.zeros((128, N_TILES), np.float32),
                "iota": _get_iota(),
            }
            for _ in range(N_CORES)
        ]
        run_bass_kernel_spmd(_get_nc(), _zero_maps, core_ids=list(range(N_CORES)))
        del _zero_maps
    except Exception as _e:  # noqa: BLE001 — warmup is best-effort only
        sys.stderr.write(f"kernel warmup skipped: {_e}\n")


# revision 4
# speedup vs baseline: 1.0168x; 1.0168x over previous
"""Trainium2 kernel for nn_Linter_89000312307760 (segment_reduce).

Pipeline (zero host-side passes over the 128 MB feature tensor):
  host:   key = m*label + index per pixel (int32 -> f32, exact); counts via
          bincount. Features are shipped as raw f32 views.
  device: (8 cores, data-parallel: core = image*4 + quarter)
          - DMA feat [64, 65536] f32 in chunks (d-major, contiguous)
          - PE transpose [64, 128] -> PSUM [128, 64]; ACT drains PSUM to
            SBUF as fp16 (transpose + downcast fused into engines that are
            otherwise idle)
          - one-hot per pixel-tile via tensor_scalar(iota == key) on POOL/DVE
          - segment sums via PSUM-accumulated matmuls: psum[64, 641] +=
            featT[128, 64]^T x onehot[128, 641] over all 512 tiles
          - one [64, 641] f32 output per core.
  host:   sum the 4 quarter outputs per image; per-class pairwise mean-|.|
          loss via an O(D n log n) sorted-prefix-sum identity (not the O(n^2 D)
          pairwise matrix); final -log scalar.

The Bass module is input-independent: it is built once at import and reused
across calls; the first dispatch is warmed at import as well.
"""
import os
import sys
import time

import numpy as np

if "/opt/trn_rl_repo" not in sys.path:
    sys.path.insert(0, "/opt/trn_rl_repo")

import bass_rust
import concourse.bass as bass
import concourse.tile as tile
from concourse import mybir
from concourse.bass_utils import run_bass_kernel_spmd
from concourse.vector_clock import ScopedClock

# ---- problem constants (hardcoded per spec) ----
B, D, H, W = 2, 64, 512, 512
P = H * W                    # pixels per image
N_CLASSES = 5
IGNORE_LB = 255
S = N_CLASSES * 128 + 1      # 641 static segment capacity
N_CORES = 8
QUARTER = P // 4             # pixels per core chunk
N_TILES = QUARTER // 128     # 512 pixel-tiles per core
CHUNK_TILES = 32             # tiles per feat DMA chunk

LAST_RUN_WALL_S = None       # wall-clock of the device execute (set per call)


# ---------------------------------------------------------------- drain patch
def _patched_drain_and_barrier(self, tick_clock, wait_clock):
    # walrus CTRL ops encode only one sync wait; the stock kernel-tail drain
    # carries one wait per logical processor. Spread them over SP nops.
    nc = self.nc
    probe = nc.sync.nop(nofuse=True, hint="drain_wait_probe")
    wait_clock.add_sem_waits(probe.ins, ScopedClock({None: tick_clock.global_clock}))
    waits = list(probe.ins.sync_info.on_wait) if probe.ins.sync_info else []
    if len(waits) > 1:
        probe.ins.sync_info = bass_rust.SyncInfo(on_wait=waits[:1], on_update=[])
        for i, w in enumerate(waits[1:]):
            n = nc.sync.nop(nofuse=True, hint=f"drain_wait_{i}")
            n.ins.sync_info = bass_rust.SyncInfo(on_wait=[w], on_update=[])
    nc.sync.drain()
    nc.all_engine_barrier()
    assert self.sems is not None
    popped = nc._tile_sem_poison_stack.pop()
    assert popped is self._sem_poison
    nc.clear_and_free_semaphores(list(self.sems.allocated().values()))
    nc.all_engine_barrier()


tile.TileContext._drain_and_barrier = _patched_drain_and_barrier

_WSPLIT_N = 0


def _split_sync_waits(nc: bass.Bass):
    """walrus encodes at most one sync wait per instruction on this target;
    move extra waits onto same-engine nops inserted immediately before."""
    global _WSPLIT_N
    for f in nc.m.functions:
        for bb in f.blocks:
            out = []
            changed = False
            for ins in bb.instructions:
                si = ins.sync_info
                if si is not None and si.on_wait and len(si.on_wait) > 1:
                    changed = True
                    waits = list(si.on_wait)
                    for w in waits[:-1]:
                        _WSPLIT_N += 1
                        out.append(
                            mybir.InstNoOp(
                                name=f"WSPLIT-{_WSPLIT_N}",
                                engine=ins.engine,
                                bass_nofuse=True,
                                sync_info=mybir.SyncInfo(on_wait=[w], on_update=[]),
                            )
                        )
                    ins.sync_info = mybir.SyncInfo(
                        on_wait=[waits[-1]], on_update=list(si.on_update)
                    )
                out.append(ins)
            if changed:
                bb.instructions = out


# ---------------------------------------------------------------- device part
def build_device_kernel(
    n_tiles: int = N_TILES,
    chunk_tiles: int = CHUNK_TILES,
    fc_bufs: int = 3,
    ft_bufs: int = 6,
    oh_bufs: int = 8,
    pst_bufs: int = 4,
) -> bass.Bass:
    nc = bass.Bass("TRN2")
    f16 = mybir.dt.float16
    f32 = mybir.dt.float32

    feat_d = nc.declare_dram_parameter("feat", [64, n_tiles * 128], f32, isOutput=False)
    keys_d = nc.declare_dram_parameter("keys", [128, n_tiles], f32, isOutput=False)
    iota_d = nc.declare_dram_parameter("iota", [128, S], f16, isOutput=False)
    ident_d = nc.declare_dram_parameter("ident", [64, 64], f32, isOutput=False)
    out_d = nc.declare_dram_parameter("out", [64, S], f32, isOutput=True)

    n_chunks = (n_tiles + chunk_tiles - 1) // chunk_tiles

    with tile.TileContext(nc) as tc:
        with (
            tc.tile_pool(name="const", bufs=1) as const_tp,
            tc.tile_pool(name="fc", bufs=fc_bufs) as fc_tp,
            tc.tile_pool(name="ft", bufs=ft_bufs) as ft_tp,
            tc.tile_pool(name="oh", bufs=oh_bufs) as oh_tp,
            tc.tile_pool(name="o", bufs=1) as out_tp,
            tc.tile_pool(name="ps", bufs=1, space="PSUM") as ps_tp,
            tc.tile_pool(name="pst", bufs=pst_bufs, space="PSUM") as pst_tp,
        ):
            iota_sb = const_tp.tile([128, S], f16)
            nc.sync.dma_start(out=iota_sb[:], in_=iota_d[:])
            keys_sb = const_tp.tile([128, n_tiles], f32)
            nc.sync.dma_start(out=keys_sb[:], in_=keys_d[:])
            ident_sb = const_tp.tile([64, 64], f32)
            nc.sync.dma_start(out=ident_sb[:], in_=ident_d[:])

            psA = ps_tp.tile([64, 512], f32, space="PSUM")
            psB = ps_tp.tile([64, S - 512], f32, space="PSUM")

            for c in range(n_chunks):
                t0 = c * chunk_tiles
                t1 = min(t0 + chunk_tiles, n_tiles)
                fchunk = fc_tp.tile([64, (t1 - t0) * 128], f32, tag="fc")
                nc.sync.dma_start(
                    out=fchunk[:], in_=feat_d[:, t0 * 128 : t1 * 128]
                )
                for t in range(t0, t1):
                    lt = t - t0
                    pst = pst_tp.tile([128, 64], f32, space="PSUM", tag="pst")
                    nc.tensor.transpose(
                        out=pst[:],
                        in_=fchunk[:, lt * 128 : (lt + 1) * 128],
                        identity=ident_sb[:],
                    )
                    ft16 = ft_tp.tile([128, 64], f16, tag="ft")
                    nc.scalar.activation(
                        out=ft16[:], in_=pst[:],
                        func=mybir.ActivationFunctionType.Copy,
                    )
                    oh = oh_tp.tile([128, S], f16, tag="oh")
                    eng = nc.gpsimd if (t % 2 == 0) else nc.vector
                    eng.tensor_scalar(
                        out=oh[:],
                        in0=iota_sb[:],
                        scalar1=keys_sb[:, t : t + 1],
                        scalar2=None,
                        op0=mybir.AluOpType.is_equal,
                    )
                    nc.tensor.matmul(
                        out=psA[:],
                        lhsT=ft16[:],
                        rhs=oh[:, 0:512],
                        start=(t == 0),
                        stop=(t == n_tiles - 1),
                    )
                    nc.tensor.matmul(
                        out=psB[:],
                        lhsT=ft16[:],
                        rhs=oh[:, 512:S],
                        start=(t == 0),
                        stop=(t == n_tiles - 1),
                    )

            out_sb = out_tp.tile([64, S], f32)
            nc.scalar.activation(
                out=out_sb[:, 0:512], in_=psA[:],
                func=mybir.ActivationFunctionType.Copy,
            )
            nc.scalar.activation(
                out=out_sb[:, 512:S], in_=psB[:],
                func=mybir.ActivationFunctionType.Copy,
            )
            nc.sync.dma_start(out=out_d[:], in_=out_sb[:])

    _split_sync_waits(nc)
    return nc


_NC = None


def _get_nc() -> bass.Bass:
    global _NC
    if _NC is None:
        _NC = build_device_kernel()
    return _NC


# ------------------------------------------------------------------ host part
_IOTA = None
_IDENT = None


def _get_iota() -> np.ndarray:
    global _IOTA
    if _IOTA is None:
        _IOTA = np.ascontiguousarray(
            np.broadcast_to(np.arange(S, dtype=np.float16), (128, S))
        )
    return _IOTA


def _get_ident() -> np.ndarray:
    global _IDENT
    if _IDENT is None:
        _IDENT = np.eye(64, dtype=np.float32)
    return _IDENT


def _host_prep(feature_out, labels, indexes):
    """Keys + counts; builds per-core in_maps (all views, no feature pass)."""
    lab = np.asarray(labels).reshape(B, P)
    idx = np.asarray(indexes).reshape(B, P)

    m = idx.max(axis=1)                                   # per-image max index
    ig = lab == IGNORE_LB
    keys = np.where(ig, 0, m[:, None] * np.where(ig, 0, lab) + np.where(ig, 0, idx))
    keys = keys.astype(np.int32)                          # [B, P] in [0, S)
    counts = np.stack([np.bincount(keys[b], minlength=S) for b in range(B)])
    keys32 = keys.astype(np.float32)

    feat = np.asarray(feature_out, dtype=np.float32).reshape(B, D, P)

    iota = _get_iota()
    ident = _get_ident()
    in_maps = []
    for core in range(N_CORES):
        b, q = divmod(core, 4)
        lo = q * QUARTER
        in_maps.append(
            {
                "feat": feat[b][:, lo : lo + QUARTER],
                "keys": keys32[b, lo : lo + QUARTER].reshape(N_TILES, 128).T,
                "iota": iota,
                "ident": ident,
            }
        )
    return in_maps, counts, m


def _phase2(sums_b, counts_b, m_b):
    """Per-image pairwise class loss via sorted prefix sums, O(D n log n).

    For one feature dim d with values v sorted ascending and class labels c:
      sum_{j in class b} |v_k - v_j|
        = v_k*(2*cnt_b(<=k) - tot_b) + sum_b(tot) - 2*sum_b(<=k)
    accumulated into P[class(k), b], then summed over d.

    sums_b: [S, D] f32 segment sums; counts_b: [S] int; m_b: max index.
    """
    cnt = counts_b.astype(np.float32)
    valid = cnt >= 2.0
    valid[0] = False
    iv = np.flatnonzero(valid)
    if iv.size == 0:
        return 0.0, 0.0
    mv = sums_b[iv].astype(np.float32) / cnt[iv, None]    # [n, D] means
    cls = (np.ceil(iv.astype(np.float64) / float(m_b)) - 1.0).astype(np.int64)

    order = np.argsort(mv, axis=0)
    w = np.take_along_axis(mv, order, 0)                  # [n, D] col-sorted
    csort = cls[order]                                    # [n, D]
    masks = [(csort == c) for c in range(N_CLASSES)]

    pair = np.zeros((N_CLASSES, N_CLASSES))
    for b in range(N_CLASSES):
        mb = masks[b]
        cntb = np.cumsum(mb, axis=0, dtype=np.float32)
        sumb = np.cumsum(np.where(mb, w, np.float32(0.0)), axis=0, dtype=np.float64)
        term = w * (2.0 * cntb - cntb[-1][None, :]) + (sumb[-1][None, :] - 2.0 * sumb)
        for a in range(N_CLASSES):
            pair[a, b] = np.sum(term, where=masks[a], dtype=np.float64)
    pair /= float(D)

    n_c = np.bincount(cls, minlength=N_CLASSES).astype(np.float64)
    npair = np.outer(n_c, n_c)
    ret = pair / np.maximum(npair, 1.0)
    h = np.where(ret < 1.0, 0.5 * ret * ret, ret - 0.5)
    tri = np.triu(np.ones((N_CLASSES, N_CLASSES)), k=1)
    pv = tri * (npair > 0.0)
    return float((h * pv).sum()), float(pv.sum())


def kernel(feature_out, labels, indexes):
    global LAST_RUN_WALL_S
    in_maps, counts, m = _host_prep(feature_out, labels, indexes)

    nc = _get_nc()
    t0 = time.monotonic()
    res = run_bass_kernel_spmd(nc, in_maps, core_ids=list(range(N_CORES)))
    LAST_RUN_WALL_S = time.monotonic() - t0

    tot_s = tot_c = 0.0
    for b in range(B):
        sums = (
            res.results[4 * b + 0]["out"]
            + res.results[4 * b + 1]["out"]
            + res.results[4 * b + 2]["out"]
            + res.results[4 * b + 3]["out"]
        ).T                                               # [S, D] f32
        s_img, c_img = _phase2(sums, counts[b], int(m[b]))
        tot_s += s_img
        tot_c += c_img

    mean_h = tot_s / max(tot_c, 1.0)
    mean_h = max(mean_h, 1e-12)
    out = -np.log(mean_h / float(B)) if tot_c > 0 else 0.0
    return np.array([out], dtype=np.float32)


# Build the module at import; warm the dispatch path (jit trace + NEFF cache +
# program load) so the first timed kernel() call pays only transfer + exec.
if os.environ.get("KERNEL_NO_WARM", "") != "1":
    try:
        _get_nc()
        _zero_maps = [
            {
                "feat": np.zeros((64, QUARTER), np.float32),
                "keys": np.zeros((128, N_TILES), np.float32),
                "iota": _get_iota(),
                "ident": _get_ident(),
            }
            for _ in range(N_CORES)
        ]
        run_bass_kernel_spmd(_get_nc(), _zero_maps, core_ids=list(range(N_CORES)))
        del _zero_maps
    except Exception as _e:  # noqa: BLE001 — warmup is best-effort only
        sys.stderr.write(f"kernel warmup skipped: {_e}\n")
